# revision 23
# baseline (speedup 1.0000x reference)
"""Trainium2 Bass kernel for nn_GATModel (GATv2 on a bidirectional chain graph).

Key algebraic facts exploited (derived from the reference):
  * The reference's conv loop feeds x0 into EVERY layer, so only the LAST
    GATv2 layer (index L-1) affects the output.
  * x0 = x @ W_exp + b_exp + pe  never needs materializing:
        xl = x0 @ Wl + bl = x @ (W_exp@Wl) + [(b_exp+pe[n])@Wl + bl]
    i.e. a [64,256] matmul plus a per-node (n) bias.
  * The graph is a chain + self loops, so message passing is a 3-tap stencil
    (left / self / right) with a masked 3-way softmax per node.
  * a . leaky_relu(z) = 0.6*(a . z) + 0.4*(a . |z|)   (slope 0.2)
    and with ahat=|a| folded into the weight columns (positively homogeneous),
    a_h*|z_h| = sign(a_h)*|ztilde_h|.  So the nonlinear part is a signed sum
    of |ztilde| and the linear part is two per-node scalars (p, q).

Device computes (per 128-row tiles, col-major [h-part, row-free]):
  z_sigma (PSUM, via matmul accumulation incl. one-hot bias matmul)
  -> |z| (DVE tensor_scalar abs_max / ACT activation Abs, PSUM->SBUF bf16)
  -> t_sigma = sum_h sign(a_h)*|ztilde| (PE matmul with sign vector)
  plus p = x@ (Wl'a), q = x @ (Wr'a), y = x @ (Wl' W_fc)  (one small matmul).
Host finishes: logits, masks, 3-way softmax, alpha-weighted message pooling,
final fc.  (All heavy O(B*N*H) math is on device.)
"""

import os
import sys

sys.path.insert(0, "/opt/trn_rl_repo")

from contextlib import ExitStack  # noqa: E402

import ml_dtypes  # noqa: E402
import numpy as np  # noqa: E402

import concourse.bass as bass  # noqa: E402
import concourse.tile as tile  # noqa: E402
from concourse import bacc, mybir  # noqa: E402
from concourse.bass_utils import run_bass_kernel_spmd  # noqa: E402

BF16 = mybir.dt.bfloat16
F32 = mybir.dt.float32
NPBF16 = ml_dtypes.bfloat16

B, N, IN, H, L, C = 2048, 100, 64, 256, 3, 3
NEG = 0.2
NCORES = 8
BC = B // NCORES            # 256 graphs per core
ROWS = BC * N               # 25600 rows per core
CH_ELEMS = 5
CHF = CH_ELEMS * N          # 500 rows per chunk
NFULL = BC // CH_ELEMS      # 51 full chunks
REM_ELEMS = BC - NFULL * CH_ELEMS   # 1 leftover graph
CHUNKS = [(i * CHF, CHF) for i in range(NFULL)]
if REM_ELEMS:
    CHUNKS.append((NFULL * CHF, REM_ELEMS * N))

LAST_RESULTS = None  # set by kernel() for test harness inspection
import os as _os
V2_TMM = _os.environ.get("V2_TMM", "1") == "1"
V2_SDMA = _os.environ.get("V2_SDMA", "1") == "1"



def _make_pe_np(n, d):
    pos = np.arange(n, dtype=np.float32)[:, None]
    div = np.exp(
        np.arange(0, d, 2, dtype=np.float32) * (-np.log(np.float32(10000.0)) / d)
    )
    pe = np.zeros((n, d), dtype=np.float32)
    pe[:, 0::2] = np.sin(pos * div)
    pe[:, 1::2] = np.cos(pos * div)
    return pe


def _route_is_dve(sigma, chunk_idx):
    # Balance the PSUM->SBUF abs pass between VectorE (4/9) and ScalarE (5/9).
    return ((sigma + 3 * chunk_idx) % 9) < 4


_PROG_CACHE = None


def _build_program():
    """Build the (shape-only) Bass program once; weights arrive via in_maps."""
    nc = bacc.Bacc(
        "TRN2",
        target_bir_lowering=False,
        debug=False,
        enable_asserts=False,
        num_devices=NCORES,
    )

    d_in = {}

    def din(name, shape, dt):
        d_in[name] = nc.dram_tensor(name, list(shape), dt, kind="ExternalInput").ap()
        return d_in[name]

    xT = din("xT", (64, ROWS), BF16)
    S_lr0 = din("S_lr0", (128, 128), BF16)
    S_lr1 = din("S_lr1", (128, 128), BF16)
    S_rl0 = din("S_rl0", (128, 128), BF16)
    S_rl1 = din("S_rl1", (128, 128), BF16)
    S_self = din("S_self", (128, 128), BF16)
    Wpqy = din("Wpqy", (128, 8), BF16)
    COEF = din("COEF", (128, 2), F32)
    # rank-64 factorized per-node biases: Dst rows0:64 = blk0 stationary,
    # rows64:128 = blk1; Bm = basis moving tile (n-periodic), duplicated
    # on partitions 64:128 so the blk1 matmul can row-tile concurrently.
    Bm_dram = {s: din(f"Bm_{s}", (128, CHF), BF16) for s in ("l", "r", "s")}
    Dst_dram = {s: din(f"Dst_{s}", (128, 128), BF16) for s in ("l", "r", "s")}
    outsT_dram = nc.dram_tensor("outsT", [8, ROWS], F32, kind="ExternalOutput").ap()
    outsP_dram = nc.dram_tensor("outsP", [5, ROWS], F32, kind="ExternalOutput").ap()

    with tile.TileContext(nc) as tc, ExitStack() as ctx:
        cpool = ctx.enter_context(tc.tile_pool(name="consts", bufs=1))
        x3pool = ctx.enter_context(tc.tile_pool(name="x3", bufs=1))
        zpool = ctx.enter_context(
            tc.tile_pool(name="z", bufs=1, space=bass.MemorySpace.PSUM)
        )
        tbpool = ctx.enter_context(
            tc.tile_pool(name="tb", bufs=1, space=bass.MemorySpace.PSUM)
        )
        wpool = ctx.enter_context(tc.tile_pool(name="w", bufs=2))
        spool = ctx.enter_context(tc.tile_pool(name="stage", bufs=2))

        def cload(name, dram_ap, shape, dt):
            t = cpool.tile(list(shape), dt, tag=f"c_{name}")
            nc.sync.dma_start(t[:], dram_ap[:])
            return t

        S_lr = [cload("slr0", S_lr0, (128, 128), BF16),
                cload("slr1", S_lr1, (128, 128), BF16)]
        S_rl = [cload("srl0", S_rl0, (128, 128), BF16),
                cload("srl1", S_rl1, (128, 128), BF16)]
        S_sf = cload("ssf", S_self, (128, 128), BF16)
        Wpq = cload("wpqy", Wpqy, (128, 8), BF16)
        CO = cload("coef", COEF, (128, 2), F32)
        Bm = {s: cload(f"bm{s}", v, (128, CHF), BF16) for s, v in Bm_dram.items()}
        Dst = {s: cload(f"dst{s}", v, (128, 128), BF16) for s, v in Dst_dram.items()}

        # x3: [0:64, c] = xT[:, c-1] (shifted), [64:128, c] = xT[:, c]
        x3 = x3pool.tile([128, ROWS + 2], BF16)
        nc.vector.memset(x3[:, 0:1], 0.0)
        nc.vector.memset(x3[:, ROWS : ROWS + 2], 0.0)
        nc.sync.dma_start(x3[64:128, 0:ROWS], xT[:, :])
        nc.sync.dma_start(x3[0:64, 1 : ROWS + 1], xT[:, :])

        for ci, (c0, F) in enumerate(CHUNKS):
            FC = F // 4  # per-col-strip t/pq chunk width
            zt = {}
            # ---- z production ----
            # Per bank: rank-64 bias matmul (start=True) + data matmul.
            # The two blk bias matmuls of one sigma row-tile concurrently
            # (rows 0:64 vs 64:128, different target banks).
            for b in (0, 1):
                zl = zpool.tile([128, F], F32, tag=f"zl{b}")
                nc.tensor.matmul(zl[:], Dst["l"][b * 64 : b * 64 + 64, :],
                                 Bm["l"][b * 64 : b * 64 + 64, 0:F],
                                 start=True, stop=False)
                nc.tensor.matmul(zl[:], S_lr[b][:], x3[:, c0 : c0 + F],
                                 start=False, stop=True)
                zt[("l", b)] = zl
            for b in (0, 1):
                zr = zpool.tile([128, F], F32, tag=f"zr{b}")
                nc.tensor.matmul(zr[:], Dst["r"][b * 64 : b * 64 + 64, :],
                                 Bm["r"][b * 64 : b * 64 + 64, 0:F],
                                 start=True, stop=False)
                nc.tensor.matmul(zr[:], S_rl[b][:], x3[:, c0 + 1 : c0 + F + 1],
                                 start=False, stop=True)
                zt[("r", b)] = zr
            zs0 = zpool.tile([128, F], F32, tag="zs0")
            nc.tensor.matmul(zs0[:], Dst["s"][0:64, :], Bm["s"][0:64, 0:F],
                             start=True, stop=False)
            nc.tensor.matmul(zs0[:], S_sf[0:64, :], x3[0:64, c0 + 1 : c0 + F + 1],
                             start=False, stop=True)
            zt[("s", 0)] = zs0
            zs1 = zpool.tile([128, F], F32, tag="zs1")
            nc.tensor.matmul(zs1[:], Dst["s"][64:128, :], Bm["s"][64:128, 0:F],
                             start=True, stop=False)
            nc.tensor.matmul(zs1[:], S_sf[64:128, :], x3[64:128, c0 : c0 + F],
                             start=False, stop=True)
            zt[("s", 1)] = zs1

            # ---- |z| crossing PSUM -> SBUF (bf16), split DVE / ACT ----
            wt = {}
            for si, s in enumerate(("l", "r", "s")):
                for b in (0, 1):
                    w = wpool.tile([128, F], F32, tag=f"w{s}{b}")
                    if _route_is_dve(si, ci):
                        # |z| = clear fp32 sign bit (exact, 1x from PSUM)
                        nc.vector.tensor_scalar(
                            w[:].bitcast(mybir.dt.int32),
                            zt[(s, b)][:].bitcast(mybir.dt.int32),
                            0x7FFFFFFF,
                            None,
                            mybir.AluOpType.bitwise_and,
                        )
                    else:
                        nc.scalar.activation(
                            w[:], zt[(s, b)][:], mybir.ActivationFunctionType.Abs
                        )
                    wt[(s, b)] = w

            if not V2_TMM:
                # legacy path: M=1 t-matmuls + pq at partitions 96..100
                tb = tbpool.tile([128, F], F32, tag="tbA")
                for si, s in enumerate(("l", "r", "s")):
                    p0 = 32 * si
                    nc.tensor.matmul(tb[p0 : p0 + 1, 0:F], CO[:, 0:1],
                                     wt[(s, 0)][:], start=True, stop=False)
                    nc.tensor.matmul(tb[p0 : p0 + 1, 0:F], CO[:, 1:2],
                                     wt[(s, 1)][:], start=False, stop=True)
                nc.tensor.matmul(tb[96:101, 0:F], Wpq[64:128, 0:5],
                                 x3[64:128, c0 : c0 + F], start=True, stop=True,
                                 tile_position=(64, 96))
                st = spool.tile([128, F], F32, tag="stA")
                if ci % 2 == 0:
                    nc.vector.tensor_copy(st[:], tb[:, 0:F])
                else:
                    nc.scalar.copy(st[:], tb[:, 0:F])
                nc.sync.dma_start(outsT_dram[0:3, c0 : c0 + F], st[0:96:32, 0:F])
                nc.sync.dma_start(outsP_dram[:, c0 : c0 + F], st[96:101, 0:F])
                continue
            # ---- t_sigma = sum_h sign(a_h) * |ztilde| ----
            # 8 concurrent PE tiles: r in {0,1} K=64-halves -> banks tbA/tbB,
            # c in {0..3} F-chunks -> 32-aligned col strips.  Host sums the
            # two K-half partials.  Only the first matmul per bank uses
            # start=True (whole-bank has_written clear).
            tbA = tbpool.tile([128, 4 * FC], F32, tag="tbA")
            tbB = tbpool.tile([128, 4 * FC], F32, tag="tbB")
            tbb = [tbA, tbB]
            started = [False, False]
            for si, s in enumerate(("l", "r", "s")):
                for b in (0, 1):          # h-block round (accumulate)
                    for r in (0, 1):      # K-half -> bank
                        for c in range(4):
                            out = tbb[r][32 * c : 32 * c + 1,
                                         si * FC : si * FC + FC]
                            nc.tensor.matmul(
                                out,
                                CO[64 * r : 64 * r + 64, b : b + 1],
                                wt[(s, b)][64 * r : 64 * r + 64,
                                           c * FC : (c + 1) * FC],
                                start=(b == 0 and not started[r]),
                                stop=(b == 1),
                                tile_position=(64 * r, 32 * c),
                                skip_group_check=True,
                            )
                            started[r] = True
            # ---- p, q, y rows: 4 F-chunks into tbB strips at offset 3*FC ----
            for c in range(4):
                nc.tensor.matmul(
                    tbb[1][32 * c : 32 * c + 5, 3 * FC : 4 * FC],
                    Wpq[64:128, 0:5],
                    x3[64:128, c0 + c * FC : c0 + (c + 1) * FC],
                    start=False, stop=True,
                    tile_position=(64, 32 * c),
                    skip_group_check=True,
                )

            # ---- evacuate + DMA out ----
            stA = spool.tile([128, 4 * FC], F32, tag="stA")
            stB = spool.tile([128, 4 * FC], F32, tag="stB")
            if ci % 2 == 0:
                nc.vector.tensor_copy(stA[:], tbb[0][:, 0 : 4 * FC])
                nc.scalar.copy(stB[:], tbb[1][:, 0 : 4 * FC])
            else:
                nc.scalar.copy(stA[:], tbb[0][:, 0 : 4 * FC])
                nc.vector.tensor_copy(stB[:], tbb[1][:, 0 : 4 * FC])
            # t partials: rows 0..3 <- bank A strips, rows 4..7 <- bank B
            nc.sync.dma_start(outsT_dram[0:4, c0 : c0 + 3 * FC],
                              stA[0:97:32, 0 : 3 * FC])
            nc.sync.dma_start(outsT_dram[4:8, c0 : c0 + 3 * FC],
                              stB[0:97:32, 0 : 3 * FC])
            for c in range(4):
                nc.sync.dma_start(
                    outsP_dram[:, c0 + c * FC : c0 + (c + 1) * FC],
                    stB[32 * c : 32 * c + 5, 3 * FC : 4 * FC])

    nc.compile()
    return nc


def _get_program():
    global _PROG_CACHE
    if _PROG_CACHE is None:
        _PROG_CACHE = _build_program()
    return _PROG_CACHE


def kernel(x, W_exp, b_exp, W_l, b_l, W_r, b_r, att, bias, W_fc, b_fc):
    global LAST_RESULTS
    x = np.asarray(x, dtype=np.float32)
    W_exp = np.asarray(W_exp, np.float32)
    b_exp = np.asarray(b_exp, np.float32)
    W_l = np.asarray(W_l, np.float32)
    b_l = np.asarray(b_l, np.float32)
    W_r = np.asarray(W_r, np.float32)
    b_r = np.asarray(b_r, np.float32)
    att = np.asarray(att, np.float32)
    bias = np.asarray(bias, np.float32)
    W_fc = np.asarray(W_fc, np.float32)
    b_fc = np.asarray(b_fc, np.float32)

    lw = L - 1  # only the last conv layer matters
    pe = _make_pe_np(N, H)
    a = att[lw]
    s = np.where(a >= 0.0, 1.0, -1.0).astype(np.float32)
    ahat = np.abs(a)

    Wl_full = W_exp @ W_l[lw]                     # [64,256]
    Wr_full = W_exp @ W_r[lw]
    cl = (b_exp + pe) @ W_l[lw] + b_l[lw]         # [100,256]
    cr = (b_exp + pe) @ W_r[lw] + b_r[lw]

    Wtl = Wl_full * ahat[None, :]                 # ahat-folded
    Wtr = Wr_full * ahat[None, :]
    ctl = cl * ahat[None, :]
    ctr = cr * ahat[None, :]

    # stationaries [K,M]: K = concat feature dim, M = h-block columns
    def blk(Wm, b):
        return Wm[:, b * 128 : (b + 1) * 128]

    def bf(arr):
        return np.ascontiguousarray(arr.astype(NPBF16))

    consts = {}
    for b in (0, 1):
        consts[f"S_lr{b}"] = bf(np.concatenate([blk(Wtl, b), blk(Wtr, b)], axis=0))
        consts[f"S_rl{b}"] = bf(np.concatenate([blk(Wtr, b), blk(Wtl, b)], axis=0))
    Wts = Wtl + Wtr
    consts["S_self"] = bf(np.concatenate([blk(Wts, 0), blk(Wts, 1)], axis=0))

    # Per-dst-node z~ biases, rank-64 factorized (pe has numerical rank ~49,
    # so rank 64 is exact to fp32 precision): D = Bfac @ Wfac
    ctl_m1 = np.vstack([np.zeros((1, H), np.float32), ctl[:-1]])   # ctl[n-1]
    ctl_p1 = np.vstack([ctl[1:], np.zeros((1, H), np.float32)])    # ctl[n+1]
    Dfull = {
        "l": ctl_m1 + ctr,
        "r": ctl_p1 + ctr,
        "s": ctl + ctr,
    }
    n_pat = np.arange(CHF) % 100
    for sname, Dm in Dfull.items():
        U, S, Vt = np.linalg.svd(Dm.astype(np.float64), full_matrices=False)
        k = 64
        rs = np.sqrt(S[:k])
        Bfac = (U[:, :k] * rs[None, :]).astype(np.float32)   # [100, 64]
        Wfac = (rs[:, None] * Vt[:k]).astype(np.float32)     # [64, 256]
        BmT = Bfac.T[:, n_pat]                               # [64, CHF]
        consts[f"Bm_{sname}"] = bf(np.concatenate([BmT, BmT], axis=0))
        consts[f"Dst_{sname}"] = bf(
            np.concatenate([Wfac[:, 0:128], Wfac[:, 128:256]], axis=0)
        )

    # p/q/y weights: [64, 5] at partitions 64:128 of a [128,8] tile
    wp = Wl_full @ a                                # [64]
    wq = Wr_full @ a
    Wy = Wl_full @ W_fc                             # [64,3]
    Wpqy = np.zeros((128, 8), np.float32)
    Wpqy[64:, 0] = wp
    Wpqy[64:, 1] = wq
    Wpqy[64:, 2:5] = Wy
    consts["Wpqy"] = bf(Wpqy)

    COEF = np.zeros((128, 2), np.float32)
    COEF[:, 0] = s[0:128]
    COEF[:, 1] = s[128:256]
    consts["COEF"] = np.ascontiguousarray(COEF)



    # per-core inputs
    xr = x.reshape(NCORES, ROWS, IN)
    in_maps = []
    for c in range(NCORES):
        m = dict(consts)
        m["xT"] = bf(xr[c].T)                      # [64, ROWS]
        in_maps.append(m)

    nc = _get_program()
    res = run_bass_kernel_spmd(
        nc,
        in_maps,
        core_ids=list(range(NCORES)),
    )
    LAST_RESULTS = res

    # ---------------- host tail ----------------
    cp = cl @ a                                               # [100]
    cq = cr @ a
    cy = cl @ W_fc                                            # [100,3]
    n_of_r = np.tile(np.arange(N), BC)                        # [ROWS]

    out_all = np.empty((B, C), np.float32)
    for c in range(NCORES):
        oT = np.asarray(res.results[c]["outsT"], np.float32)  # [8, ROWS]
        oP = np.asarray(res.results[c]["outsP"], np.float32)  # [5, ROWS]
        if not V2_TMM:
            t_all = oT[0:3]
        else:
            # decode t partials: row 4r+cs holds [sigma*FC + j'] per F-chunk
            t_all = np.empty((3, ROWS), np.float32)
            for c0, Fc in CHUNKS:
                FC = Fc // 4
                blk = (oT[0:4, c0 : c0 + 3 * FC]
                       + oT[4:8, c0 : c0 + 3 * FC])           # [4, 3*FC]
                blk = blk.reshape(4, 3, FC)                    # [cs, sigma, j']
                t_all[:, c0 : c0 + Fc] = (
                    blk.transpose(1, 0, 2).reshape(3, Fc)
                )
        t_l, t_r, t_s = t_all[0], t_all[1], t_all[2]
        P, Q = oP[0], oP[1]
        Yd = oP[2:5].T                                        # [ROWS,3]

        Pb = P + cp[n_of_r]                                   # a.xl per row
        Qb = Q + cq[n_of_r]                                   # a.xr per row
        Y = Yd + cy[n_of_r]                                   # xl @ W_fc per row

        Pb_m1 = np.roll(Pb, 1)                                # P at source row r-1
        Pb_p1 = np.roll(Pb, -1)

        lg_l = 0.6 * (Pb_m1 + Qb) + 0.4 * t_l
        lg_r = 0.6 * (Pb_p1 + Qb) + 0.4 * t_r
        lg_s = 0.6 * (Pb + Qb) + 0.4 * t_s

        lg_l = np.where(n_of_r == 0, -np.inf, lg_l)
        lg_r = np.where(n_of_r == N - 1, -np.inf, lg_r)

        mx = np.maximum(np.maximum(lg_l, lg_r), lg_s)
        el = np.exp(lg_l - mx)
        er = np.exp(lg_r - mx)
        es = np.exp(lg_s - mx)
        den = el + er + es
        al, ar, asf = el / den, er / den, es / den

        Y_m1 = np.roll(Y, 1, axis=0)
        Y_p1 = np.roll(Y, -1, axis=0)
        msgs = al[:, None] * Y_m1 + ar[:, None] * Y_p1 + asf[:, None] * Y
        pooled = msgs.reshape(BC, N, C).sum(axis=1)
        out_all[c * BC : (c + 1) * BC] = (
            pooled + N * (bias[lw] @ W_fc)[None, :] + b_fc[None, :]
        )
    return out_all
